# revision 10
# baseline (speedup 1.0000x reference)
"""2-layer GATv2 (N=50000, E=1.6M, D=H=128, O=64) on 8 trn2 NeuronCores.

v2 strategy (dst-partitioned 1D, edges live with their dst owner):
- One BATCHED indirect gather per 128-dst-node block (T_b*128 edges in a
  single SWDGE instruction) from a replicated per-layer source table,
  amortizing the ~1us fixed SWDGE cost that dominated v1.
- |att| is folded into the source/target transforms host-side and the
  hidden columns are permuted so att>=0 columns come first.  With
  leaky_relu(s) = 0.2*s + 0.8*relu(s) and relu(a+b) = max(a,-b) + b, the
  attention logit reduces to e = 0.2*gdot[src] + 0.8*(sum_pos - sum_neg)
  of max(xl_hat[src], -B[dst]); all dst-side linear terms are constant
  within a softmax group and cancel.  gdot rides the gather table as an
  extra column; the max is one batched DVE op per 4/8-tile group.
- Dst expansion (-B[dst]) via one-hot matmul (tensor engine); aggregation
  via per-tile one-hot matmul accumulation with an appended ones column
  for the softmax denominator; per-node normalization multiplies back
  1/|att| and adds the skip path.
- bf16 everywhere off the critical accuracy path; PSUM->bf16 copies, exp
  and relu run on the otherwise-idle scalar (ACT) engine; one-hot builds
  and reductions alternate between vector and gpsimd (Pool) engines.
"""
import json
import sys

import ml_dtypes
import numpy as np

import concourse.bass as bass
import concourse.mybir as mybir
import concourse.tile as tile
from concourse.masks import make_identity

F32 = mybir.dt.float32
BF16 = mybir.dt.bfloat16
I32 = mybir.dt.int32
AL = mybir.AluOpType
ACTF = mybir.ActivationFunctionType

# ---------------------------------------------------------------------------
# environment fixups (walrus single-sync-wait limit)
# ---------------------------------------------------------------------------
_SPLIT_SEQ = [0]


def _split_multi_waits_json(m):
    for fn in m.get("functions", []):
        for bb in fn.get("blocks", []):
            insts = bb.get("instructions")
            if not insts:
                continue
            out = []
            for inst in insts:
                si = inst.get("sync_info")
                waits = si.get("on_wait") if si else None
                if waits and len(waits) > 1:
                    for w in waits[:-1]:
                        _SPLIT_SEQ[0] += 1
                        out.append({
                            "debug": inst.get("debug", 0),
                            "engine": inst["engine"],
                            "ins": [], "outs": [],
                            "name": f"waitsplit-{_SPLIT_SEQ[0]}",
                            "opcode": "NoOp",
                            "sync_info": {"on_update": [], "on_wait": [w]},
                        })
                    si["on_wait"] = [waits[-1]]
                out.append(inst)
            bb["instructions"] = out
    return m


_FIXED = [False]


def _install_fixups():
    if _FIXED[0]:
        return
    _FIXED[0] = True
    orig = bass.Bass.to_json_bytes

    def patched(self, *a, **k):
        return json.dumps(
            _split_multi_waits_json(json.loads(orig(self, *a, **k)))
        ).encode()

    bass.Bass.to_json_bytes = patched


# ---------------------------------------------------------------------------
# problem constants (hardcoded shapes)
# ---------------------------------------------------------------------------
N, E, D, H, O = 50000, 1_600_000, 128, 128, 64
NC, NBLK = 8, 49
NOWN = NBLK * 128           # 6272
NPAD = NC * NOWN            # 50176
F1, F2 = H, O
FW1, FW2 = F1 + 2, F2 + 2   # sbuf gall strides: [gdot | feat(F) | ones]


def _host_prep(x, edge_index, w, T_b=None):
    src, dst = np.asarray(edge_index[0]), np.asarray(edge_index[1])
    x = np.asarray(x, np.float32)
    xpad = np.zeros((NPAD, D), np.float32)
    xpad[:N] = x
    xT_b16 = np.ascontiguousarray(xpad.T).astype(ml_dtypes.bfloat16)

    per_core_blocks = []
    for c in range(NC):
        m = (dst // NOWN) == c
        s_c, d_c = src[m], dst[m] - c * NOWN
        order = np.argsort(d_c, kind="stable")
        s_c, d_c = s_c[order], d_c[order]
        bounds = np.searchsorted(d_c, np.arange(0, NOWN + 1, 128))
        per_core_blocks.append([
            (s_c[bounds[b] : bounds[b + 1]], d_c[bounds[b] : bounds[b + 1]] - b * 128)
            for b in range(NBLK)
        ])
    need = max(
        (len(s) + 127) // 128 for blocks in per_core_blocks for s, _ in blocks
    )
    if T_b is None:
        T_b = need
    assert need <= T_b
    TW = T_b * 128

    # --- attention folding: permute columns so att>=0 first, scale by |att|
    def fold(Wl, Wr, bl, br, att, blin, bskip):
        Fdim = att.shape[0]
        p = np.argsort(att < 0, kind="stable")
        P = int((att >= 0).sum())
        a = att[p]
        aa = np.abs(a)
        What = Wl[:, p] * aa                     # x_hat_l weights
        Whr = Wr[:, p] * aa                      # x_hat_r weights
        ch = aa * (bl + br)[p]                   # c_hat
        wgdot = 0.2 * (Wl @ att)                 # table gdot column
        WT = np.concatenate([wgdot[:, None], What], 1)       # [in, F+1]
        WR = -Whr                                            # [in, F]
        crow = -ch                                           # [F]
        Wlin = blin[:, p]
        srow = (bskip + bl)[p]
        inv = 1.0 / np.maximum(aa, 1e-30)
        return p, P, WT, WR, crow, Wlin, srow, inv

    p1, P1, WT1, WR1, c1row, Wlin1p, s1row, inv1 = fold(
        np.asarray(w["Wl1"], np.float32), np.asarray(w["Wr1"], np.float32),
        np.asarray(w["bl1"], np.float32), np.asarray(w["br1"], np.float32),
        np.asarray(w["att1"], np.float32), np.asarray(w["Wlin1"], np.float32),
        np.asarray(w["blin1"], np.float32) + np.asarray(w["bias1"], np.float32),
    )
    # layer 2 inputs live in layer-1 permuted space: permute rows by p1 first
    p2, P2, WT2, WR2, c2row, Wlin2p, s2row, inv2 = fold(
        np.asarray(w["Wl2"], np.float32)[p1], np.asarray(w["Wr2"], np.float32)[p1],
        np.asarray(w["bl2"], np.float32), np.asarray(w["br2"], np.float32),
        np.asarray(w["att2"], np.float32), np.asarray(w["Wlin2"], np.float32)[p1],
        np.asarray(w["blin2"], np.float32) + np.asarray(w["bias2"], np.float32),
    )

    shared = {
        "WT1": WT1.astype(ml_dtypes.bfloat16),
        "WR1": WR1.astype(ml_dtypes.bfloat16),
        "Wlin1": Wlin1p.astype(ml_dtypes.bfloat16),
        "WT2": WT2.astype(ml_dtypes.bfloat16),
        "WR2": WR2.astype(ml_dtypes.bfloat16),
        "Wlin2": Wlin2p.astype(ml_dtypes.bfloat16),
        "c1_rep": np.tile(c1row, (128, 1)).astype(np.float32),
        "s1_rep": np.tile(s1row, (128, 1)).astype(np.float32),
        "c2_rep": np.tile(c2row, (128, 1)).astype(np.float32),
        "s2_rep": np.tile(s2row, (128, 1)).astype(np.float32),
        "inv1_rep": np.tile(inv1, (128, 1)).astype(np.float32),
        "inv2_rep": np.tile(inv2, (128, 1)).astype(np.float32),
        "iota_col": np.arange(128, dtype=np.float32)[:, None],
        "iota_row": np.tile(np.arange(128, dtype=ml_dtypes.bfloat16), (128, 4)),
    }
    in_maps = []
    for c in range(NC):
        srcidx = np.zeros((NBLK, 128, T_b), np.int32)
        drel_row = np.full((NBLK, 1, TW), -1.0, np.float32)
        drel_col = np.full((NBLK, 128, T_b), -1.0, np.float32)
        for b, (s_b, drel_b) in enumerate(per_core_blocks[c]):
            n = len(s_b)
            sp = np.zeros(TW, np.int32)
            dp = np.full(TW, -1.0, np.float32)
            sp[:n] = s_b
            dp[:n] = drel_b
            srcidx[b] = sp.reshape(T_b, 128).T
            drel_col[b] = dp.reshape(T_b, 128).T
            drel_row[b, 0] = dp
        im = dict(shared)
        im["x_ownT"] = np.ascontiguousarray(
            xpad[c * NOWN : (c + 1) * NOWN].T
        ).astype(ml_dtypes.bfloat16)
        im["srcidx"] = srcidx
        im["drel_row"] = drel_row.astype(ml_dtypes.bfloat16)
        im["drel_col"] = drel_col.astype(ml_dtypes.bfloat16)
        xg = np.empty((NBLK, D, TW), ml_dtypes.bfloat16)
        for b in range(NBLK):
            flat = srcidx[b].T.ravel()              # edge order e = t*128+p
            xg[b] = xT_b16[:, flat]
        im["xgT"] = xg
        in_maps.append(im)
    return in_maps, T_b, (P1, P2, p2)


def _build_program(T_b, P1, P2):
    TW = T_b * 128
    NBT = NPAD // 128
    nc = bass.Bass()

    def din(name, shape, dt=BF16):
        return nc.dram_tensor(name, shape, dt, kind="ExternalInput")

    x_ownT = din("x_ownT", [D, NOWN])
    xgT = din("xgT", [NBLK, D, TW])
    srcidx = din("srcidx", [NBLK, 128, T_b], I32)
    drel_row = din("drel_row", [NBLK, 1, TW])
    drel_col = din("drel_col", [NBLK, 128, T_b])
    WT1 = din("WT1", [D, F1 + 1]); WR1 = din("WR1", [D, F1]); Wlin1 = din("Wlin1", [D, F1])
    WT2 = din("WT2", [H, F2 + 1]); WR2 = din("WR2", [H, F2]); Wlin2 = din("Wlin2", [H, F2])
    c1_rep = din("c1_rep", [128, F1], F32); s1_rep = din("s1_rep", [128, F1], F32)
    c2_rep = din("c2_rep", [128, F2], F32); s2_rep = din("s2_rep", [128, F2], F32)
    inv1_rep = din("inv1_rep", [128, F1], F32); inv2_rep = din("inv2_rep", [128, F2], F32)
    iota_col = din("iota_col", [128, 1], F32)
    iota_row = din("iota_row", [128, 512])
    out_own = nc.dram_tensor("out_own", [NOWN, O], F32, kind="ExternalOutput")

    with tile.TileContext(nc) as tc:
        with (
            tc.tile_pool(name="dram", bufs=1, space="DRAM") as dram,
            tc.tile_pool(name="const", bufs=1) as cpool,
            tc.tile_pool(name="res", bufs=1) as rpool,
            tc.tile_pool(name="blk", bufs=2) as bpool,
            tc.tile_pool(name="wrt", bufs=2) as wpool,
            tc.tile_pool(name="oh", bufs=2) as ohpool,
            tc.tile_pool(name="g", bufs=2) as gpool,
            tc.tile_pool(name="ga", bufs=3) as gapool,
            tc.tile_pool(name="ew", bufs=3) as epool,
            tc.tile_pool(name="sg", bufs=3) as segpool,
            tc.tile_pool(name="sm", bufs=2) as smpool,
            tc.tile_pool(name="psr", bufs=2, space="PSUM") as ppsr,
            tc.tile_pool(name="psx", bufs=2, space="PSUM") as ppsx,
            tc.tile_pool(name="pU", bufs=2, space="PSUM") as pU,
            tc.tile_pool(name="pmx", bufs=2, space="PSUM") as pmix,
        ):
            xl2_own_d = dram.tile([NOWN, F2 + 1], BF16)
            table2 = dram.tile([NPAD, F2 + 1], BF16)

            def ld(shape, apsrc, name, dt=BF16):
                t = cpool.tile(shape, dt, tag=name)
                nc.sync.dma_start(out=t[:], in_=apsrc)
                return t

            WT1_s = ld([D, F1 + 1], WT1[:], "WT1")
            WR1_s = ld([D, F1], WR1[:], "WR1")
            Wlin1_s = ld([D, F1], Wlin1[:], "Wlin1")
            WT2_s = ld([H, F2 + 1], WT2[:], "WT2")
            WR2_s = ld([H, F2], WR2[:], "WR2")
            Wlin2_s = ld([H, F2], Wlin2[:], "Wlin2")
            c1_s = ld([128, F1], c1_rep[:], "c1", F32)
            s1_s = ld([128, F1], s1_rep[:], "s1", F32)
            c2_s = ld([128, F2], c2_rep[:], "c2", F32)
            s2_s = ld([128, F2], s2_rep[:], "s2", F32)
            i1_s = ld([128, F1], inv1_rep[:], "inv1", F32)
            i2_s = ld([128, F2], inv2_rep[:], "inv2", F32)
            ic_s = ld([128, 1], iota_col[:], "ic", F32)
            ir_s = ld([128, 512], iota_row[:], "ir")
            ones1 = cpool.tile([1, 128], BF16, tag="ones1")
            nc.vector.memset(ones1[:], 1.0)
            ident = cpool.tile([128, 128], BF16, tag="ident")
            make_identity(nc, ident[:])


            # ---- R1 (-B) and skip for owned blocks ----
            R1_all = rpool.tile([128, NBLK * F1], BF16, tag="R1_all")
            skip_all = rpool.tile([128, NBLK * F1], BF16, tag="skip_all")
            hT_all = rpool.tile([128, NBLK * 128], BF16, tag="hT_all")
            for b in range(NBLK):
                xob = bpool.tile([D, 128], BF16, tag="xob")
                nc.sync.dma_start(out=xob[:], in_=x_ownT[:, b * 128 : (b + 1) * 128])
                ps = pmix.tile([128, F1 + 1], F32, tag="pmx")
                nc.tensor.matmul(out=ps[:, :F1], lhsT=xob[:], rhs=WR1_s[:], start=True, stop=True)
                nc.vector.tensor_add(
                    out=R1_all[:, b * F1 : (b + 1) * F1], in0=ps[:, :F1], in1=c1_s[:]
                )
                ps2 = pmix.tile([128, F1 + 1], F32, tag="pmx")
                nc.tensor.matmul(out=ps2[:, :F1], lhsT=xob[:], rhs=Wlin1_s[:], start=True, stop=True)
                nc.vector.tensor_add(
                    out=skip_all[:, b * F1 : (b + 1) * F1], in0=ps2[:, :F1], in1=s1_s[:]
                )

            R2_all = rpool.tile([128, NBLK * F2], BF16, tag="R2_all")
            skip2_all = rpool.tile([128, NBLK * F2], BF16, tag="skip2_all")

            def edge_pass(layer):
                F = F1 if layer == 1 else F2
                FW = F + 2
                P = P1 if layer == 1 else P2
                G = 4 if layer == 1 else 8     # tiles per psum group (G*F = 512)
                table = None if layer == 1 else table2
                R_src = R1_all if layer == 1 else R2_all
                skip_src = skip_all if layer == 1 else skip2_all
                inv_s = i1_s if layer == 1 else i2_s
                for b in range(NBLK):
                    drow = bpool.tile([1, TW], BF16, tag="drow")
                    nc.sync.dma_start(out=drow[:], in_=drel_row[b, :, :])
                    dcol = bpool.tile([128, T_b], BF16, tag="dcol")
                    nc.sync.dma_start(out=dcol[:], in_=drel_col[b, :, :])
                    if layer == 2:
                        idxb = bpool.tile([128, T_b], I32, tag="idxb")
                        nc.sync.dma_start(out=idxb[:], in_=srcidx[b, :, :])
                    else:
                        xgt = gpool.tile([D, TW], BF16, tag="xgt")
                        nc.sync.dma_start(out=xgt[:], in_=xgT[b, :, :])
                    # one-hot dst-expansion matrix oh[d, e] = (dst_e == d)
                    oh = ohpool.tile([128, TW], BF16, tag="oh")
                    for ci, c0 in enumerate(range(0, TW, 512)):
                        cw = min(512, TW - c0)
                        psr = ppsr.tile([128, 512], F32, tag="psr")
                        nc.tensor.matmul(
                            out=psr[:, :cw], lhsT=ones1[:],
                            rhs=drow[:, c0 : c0 + cw], start=True, stop=True,
                        )
                        nc.vector.tensor_scalar(
                            out=oh[:, c0 : c0 + cw], in0=psr[:, :cw],
                            scalar1=ic_s[:, :1], scalar2=None, op0=AL.is_equal,
                        )
                    # batched gather: [gdot | feat(F)] cols per edge; ones at F+1
                    gall = gapool.tile([128, T_b * FW], BF16, tag="gall")
                    g3 = gall[:].rearrange("p (t f) -> p t f", f=FW)
                    nc.vector.memset(g3[:, :, F + 1 : F + 2], 1.0)
                    if layer == 2:
                        for i in range(T_b):
                            nc.gpsimd.indirect_dma_start(
                                out=gall[:, i * FW : i * FW + F + 1], out_offset=None,
                                in_=table[:],
                                in_offset=bass.IndirectOffsetOnAxis(ap=idxb[:, i : i + 1], axis=0),
                            )
                    else:
                        for t0 in range(0, T_b, 3):
                            tq = min(3, T_b - t0)
                            pg = pmix.tile([128, 3 * (F1 + 1)], F32, tag="pmx")
                            for i in range(t0, t0 + tq):
                                nc.tensor.matmul(
                                    out=pg[:, (i - t0) * (F + 1) : (i - t0 + 1) * (F + 1)],
                                    lhsT=xgt[:, i * 128 : (i + 1) * 128],
                                    rhs=WT1_s[:], start=True, stop=True,
                                )
                            cpe = nc.scalar if (t0 // 3) % 2 == 0 else None
                            dst = g3[:, t0 : t0 + tq, 0 : F + 1]
                            srcv = pg[:, : tq * (F + 1)].rearrange(
                                "p (t f) -> p t f", f=F + 1
                            )
                            if cpe is not None:
                                nc.scalar.copy(out=dst, in_=srcv)
                            else:
                                nc.vector.tensor_copy(out=dst, in_=srcv)
                    eall = smpool.tile([128, T_b], F32, tag="eall")
                    pall = smpool.tile([128, T_b], BF16, tag="pall")
                    dbuf = smpool.tile([128, T_b], F32, tag="dbuf")
                    redp = smpool.tile([128, T_b], F32, tag="redp")
                    redn = smpool.tile([128, T_b], F32, tag="redn")
                    for g0 in range(0, T_b, G):
                        q = min(G, T_b - g0)
                        psx = ppsx.tile([128, 512], F32, tag="psx")
                        for i in range(g0, g0 + q):
                            nc.tensor.matmul(
                                out=psx[:, (i - g0) * F : (i - g0 + 1) * F],
                                lhsT=oh[:, i * 128 : (i + 1) * 128],
                                rhs=R_src[:, b * F : (b + 1) * F],
                                start=True, stop=True,
                            )
                        z = epool.tile([128, 512], BF16, tag="z")
                        nc.vector.tensor_tensor(
                            out=z[:, : q * F],
                            in0=g3[:, g0 : g0 + q, 1 : F + 1],
                            in1=psx[:, : q * F], op=AL.max,
                        )
                        z3 = z[:, : q * F].rearrange("p (t f) -> p t f", f=F)
                        nc.vector.tensor_reduce(
                            out=redp[:, g0 : g0 + q], in_=z3[:, :, :P],
                            axis=mybir.AxisListType.X, op=AL.add,
                        )
                        nc.vector.tensor_reduce(
                            out=redn[:, g0 : g0 + q], in_=z3[:, :, P:],
                            axis=mybir.AxisListType.X, op=AL.add,
                        )
                        nc.vector.tensor_sub(
                            out=dbuf[:, g0 : g0 + q],
                            in0=redp[:, g0 : g0 + q], in1=redn[:, g0 : g0 + q],
                        )
                        gcol = g3[:, g0 : g0 + q, 0:1].rearrange("p q one -> p (q one)")
                        nc.vector.scalar_tensor_tensor(
                            out=eall[:, g0 : g0 + q], in0=dbuf[:, g0 : g0 + q],
                            scalar=0.8, in1=gcol, op0=AL.mult, op1=AL.add,
                        )
                    U = pU.tile([128, F + 1], F32, tag="pU")
                    for g0 in range(0, T_b, 4):
                        q = min(4, T_b - g0)
                        nc.scalar.activation(
                            out=pall[:, g0 : g0 + q], in_=eall[:, g0 : g0 + q],
                            func=ACTF.Exp,
                        )
                        sl = dcol[:, g0 : g0 + q]
                        dbc = bass.AP(
                            sl.tensor, sl.offset,
                            [[x[0], x[1]] for x in sl.ap] + [[0, 128]],
                        )
                        sega = segpool.tile([128, 512], BF16, tag="sega")
                        nc.vector.tensor_tensor(
                            out=sega[:, : q * 128], in0=ir_s[:, : q * 128],
                            in1=dbc, op=AL.is_equal,
                        )
                        pl = pall[:, g0 : g0 + q]
                        pbc = bass.AP(
                            pl.tensor, pl.offset,
                            [[x[0], x[1]] for x in pl.ap] + [[0, 128]],
                        )
                        seg4 = segpool.tile([128, 512], BF16, tag="seg4")
                        nc.vector.tensor_tensor(
                            out=seg4[:, : q * 128], in0=sega[:, : q * 128],
                            in1=pbc, op=AL.mult,
                        )
                        for i in range(g0, g0 + q):
                            nc.tensor.matmul(
                                out=U[:], lhsT=seg4[:, (i - g0) * 128 : (i - g0 + 1) * 128],
                                rhs=gall[:, i * FW + 1 : i * FW + F + 2],
                                start=(i == 0), stop=(i == T_b - 1),
                            )
                    se = smpool.tile([128, 1], F32, tag="se")
                    nc.vector.tensor_scalar(
                        out=se[:], in0=U[:, F : F + 1], scalar1=1e-30,
                        scalar2=None, op0=AL.add,
                    )
                    r = smpool.tile([128, 1], F32, tag="r")
                    nc.vector.reciprocal(out=r[:], in_=se[:])
                    t1 = epool.tile([128, F], F32, tag="t1")
                    nc.vector.scalar_tensor_tensor(
                        out=t1[:], in0=U[:, :F], scalar=r[:, :1],
                        in1=inv_s[:], op0=AL.mult, op1=AL.mult,
                    )
                    t2 = epool.tile([128, F], F32, tag="t2")
                    nc.vector.tensor_add(
                        out=t2[:], in0=t1[:], in1=skip_src[:, b * F : (b + 1) * F]
                    )
                    if layer == 1:
                        hb = epool.tile([128, F], BF16, tag="hb")
                        nc.scalar.activation(out=hb[:], in_=t2[:], func=ACTF.Relu)
                        pst = pmix.tile([128, 128], BF16, tag="pmx")
                        nc.tensor.transpose(out=pst[:], in_=hb[:], identity=ident[:])
                        nc.scalar.copy(
                            out=hT_all[:, b * 128 : (b + 1) * 128], in_=pst[:]
                        )
                    else:
                        ob = epool.tile([128, F], F32, tag="ob")
                        nc.vector.tensor_copy(out=ob[:], in_=t2[:])
                        nc.sync.dma_start(
                            out=out_own[b * 128 : (b + 1) * 128, :], in_=ob[:]
                        )

            edge_pass(1)

            # ---- layer-2 tables from h ----
            x2w = wpool.tile([128, 2 * (F2 + 1)], BF16, tag="x2w")
            for b in range(NBLK):
                hTb = hT_all[:, b * 128 : (b + 1) * 128]
                ps = pmix.tile([128, F1 + 1], F32, tag="pmx")
                nc.tensor.matmul(out=ps[:, : F2 + 1], lhsT=hTb, rhs=WT2_s[:], start=True, stop=True)
                half = b % 2
                nc.scalar.copy(
                    out=x2w[:, half * (F2 + 1) : (half + 1) * (F2 + 1)],
                    in_=ps[:, : F2 + 1],
                )
                if half == 1:
                    blk0 = b - 1
                    nc.sync.dma_start(
                        out=xl2_own_d[blk0 * 128 : (blk0 + 2) * 128, :].rearrange(
                            "(t p) f -> p t f", t=2
                        ),
                        in_=x2w[:].rearrange("p (t f) -> p t f", f=F2 + 1),
                    )
                    x2w = wpool.tile([128, 2 * (F2 + 1)], BF16, tag="x2w")
                elif b == NBLK - 1:
                    nc.sync.dma_start(
                        out=xl2_own_d[b * 128 : (b + 1) * 128, :],
                        in_=x2w[:, : F2 + 1],
                    )
                ps2 = pmix.tile([128, F1 + 1], F32, tag="pmx")
                nc.tensor.matmul(out=ps2[:, :F2], lhsT=hTb, rhs=WR2_s[:], start=True, stop=True)
                nc.vector.tensor_add(out=R2_all[:, b * F2 : (b + 1) * F2], in0=ps2[:, :F2], in1=c2_s[:])
                ps3 = pmix.tile([128, F1 + 1], F32, tag="pmx")
                nc.tensor.matmul(out=ps3[:, :F2], lhsT=hTb, rhs=Wlin2_s[:], start=True, stop=True)
                nc.vector.tensor_add(out=skip2_all[:, b * F2 : (b + 1) * F2], in0=ps3[:, :F2], in1=s2_s[:])
            nc.gpsimd.collective_compute(
                "AllGather", AL.bypass,
                replica_groups=[list(range(NC))],
                ins=[xl2_own_d[:].opt()],
                outs=[table2[:].opt()],
            )

            edge_pass(2)

    return nc


_W_KEYS = [
    "Wl1", "bl1", "Wr1", "br1", "att1", "bias1", "Wlin1", "blin1",
    "Wl2", "bl2", "Wr2", "br2", "att2", "bias2", "Wlin2", "blin2",
]


def kernel(x, edge_index, **w):
    _install_fixups()
    from concourse.bass_utils import run_bass_kernel_spmd

    w = {k: np.asarray(w[k], np.float32) for k in _W_KEYS}
    in_maps, T_b, (P1, P2, p2) = _host_prep(np.asarray(x), np.asarray(edge_index), w)
    nc = _build_program(T_b, P1, P2)
    last_err = None
    for attempt in range(3):
        try:
            res = run_bass_kernel_spmd(nc, in_maps, core_ids=list(range(NC)))
            break
        except Exception as exc:  # flaky device recovery
            last_err = exc
            print(f"kernel: attempt {attempt} failed: {exc}", file=sys.stderr)
    else:
        raise last_err
    perm_out = np.concatenate(
        [res.results[c]["out_own"] for c in range(NC)], axis=0
    )[:N]
    out = np.empty_like(perm_out)
    out[:, p2] = perm_out
    return out.astype(np.float32)


# revision 12
# speedup vs baseline: 1.1333x; 1.1333x over previous
"""2-layer GATv2 (N=50000, E=1.6M, D=H=128, O=64) on 8 trn2 NeuronCores.

v2 strategy (dst-partitioned 1D, edges live with their dst owner):
- One BATCHED indirect gather per 128-dst-node block (T_b*128 edges in a
  single SWDGE instruction) from a replicated per-layer source table,
  amortizing the ~1us fixed SWDGE cost that dominated v1.
- |att| is folded into the source/target transforms host-side and the
  hidden columns are permuted so att>=0 columns come first.  With
  leaky_relu(s) = 0.2*s + 0.8*relu(s) and relu(a+b) = max(a,-b) + b, the
  attention logit reduces to e = 0.2*gdot[src] + 0.8*(sum_pos - sum_neg)
  of max(xl_hat[src], -B[dst]); all dst-side linear terms are constant
  within a softmax group and cancel.  gdot rides the gather table as an
  extra column; the max is one batched DVE op per 4/8-tile group.
- Dst expansion (-B[dst]) via one-hot matmul (tensor engine); aggregation
  via per-tile one-hot matmul accumulation with an appended ones column
  for the softmax denominator; per-node normalization multiplies back
  1/|att| and adds the skip path.
- bf16 everywhere off the critical accuracy path; PSUM->bf16 copies, exp
  and relu run on the otherwise-idle scalar (ACT) engine; one-hot builds
  and reductions alternate between vector and gpsimd (Pool) engines.
"""
import json
import sys

import ml_dtypes
import numpy as np

import concourse.bass as bass
import concourse.mybir as mybir
import concourse.tile as tile
from concourse.masks import make_identity

F32 = mybir.dt.float32
BF16 = mybir.dt.bfloat16
I32 = mybir.dt.int32
AL = mybir.AluOpType
ACTF = mybir.ActivationFunctionType

# ---------------------------------------------------------------------------
# environment fixups (walrus single-sync-wait limit)
# ---------------------------------------------------------------------------
_SPLIT_SEQ = [0]


def _split_multi_waits_json(m):
    for fn in m.get("functions", []):
        for bb in fn.get("blocks", []):
            insts = bb.get("instructions")
            if not insts:
                continue
            out = []
            for inst in insts:
                si = inst.get("sync_info")
                waits = si.get("on_wait") if si else None
                if waits and len(waits) > 1:
                    for w in waits[:-1]:
                        _SPLIT_SEQ[0] += 1
                        out.append({
                            "debug": inst.get("debug", 0),
                            "engine": inst["engine"],
                            "ins": [], "outs": [],
                            "name": f"waitsplit-{_SPLIT_SEQ[0]}",
                            "opcode": "NoOp",
                            "sync_info": {"on_update": [], "on_wait": [w]},
                        })
                    si["on_wait"] = [waits[-1]]
                out.append(inst)
            bb["instructions"] = out
    return m


_FIXED = [False]


def _install_fixups():
    if _FIXED[0]:
        return
    _FIXED[0] = True
    orig = bass.Bass.to_json_bytes

    def patched(self, *a, **k):
        return json.dumps(
            _split_multi_waits_json(json.loads(orig(self, *a, **k)))
        ).encode()

    bass.Bass.to_json_bytes = patched


# ---------------------------------------------------------------------------
# problem constants (hardcoded shapes)
# ---------------------------------------------------------------------------
N, E, D, H, O = 50000, 1_600_000, 128, 128, 64
NC, NBLK = 8, 49
NOWN = NBLK * 128           # 6272
NPAD = NC * NOWN            # 50176
F1, F2 = H, O
FW1, FW2 = F1 + 2, F2 + 2   # sbuf gall strides: [gdot | feat(F) | ones]


def _host_prep(x, edge_index, w, T_b=None):
    src, dst = np.asarray(edge_index[0]), np.asarray(edge_index[1])
    x = np.asarray(x, np.float32)
    xpad = np.zeros((NPAD, D), np.float32)
    xpad[:N] = x
    xT_b16 = np.ascontiguousarray(xpad.T).astype(ml_dtypes.bfloat16)

    per_core_blocks = []
    for c in range(NC):
        m = (dst // NOWN) == c
        s_c, d_c = src[m], dst[m] - c * NOWN
        order = np.argsort(d_c, kind="stable")
        s_c, d_c = s_c[order], d_c[order]
        bounds = np.searchsorted(d_c, np.arange(0, NOWN + 1, 128))
        per_core_blocks.append([
            (s_c[bounds[b] : bounds[b + 1]], d_c[bounds[b] : bounds[b + 1]] - b * 128)
            for b in range(NBLK)
        ])
    need = max(
        (len(s) + 127) // 128 for blocks in per_core_blocks for s, _ in blocks
    )
    if T_b is None:
        T_b = need
    assert need <= T_b
    TW = T_b * 128

    # --- attention folding: permute columns so att>=0 first, scale by |att|
    def fold(Wl, Wr, bl, br, att, blin, bskip):
        Fdim = att.shape[0]
        p = np.argsort(att < 0, kind="stable")
        P = int((att >= 0).sum())
        a = att[p]
        aa = np.abs(a)
        What = Wl[:, p] * aa                     # x_hat_l weights
        Whr = Wr[:, p] * aa                      # x_hat_r weights
        ch = aa * (bl + br)[p]                   # c_hat
        wgdot = 0.2 * (Wl @ att)                 # table gdot column
        WT = np.concatenate([wgdot[:, None], What], 1)       # [in, F+1]
        WR = -Whr                                            # [in, F]
        crow = -ch                                           # [F]
        Wlin = blin[:, p]
        srow = (bskip + bl)[p]
        inv = 1.0 / np.maximum(aa, 1e-30)
        return p, P, WT, WR, crow, Wlin, srow, inv

    p1, P1, WT1, WR1, c1row, Wlin1p, s1row, inv1 = fold(
        np.asarray(w["Wl1"], np.float32), np.asarray(w["Wr1"], np.float32),
        np.asarray(w["bl1"], np.float32), np.asarray(w["br1"], np.float32),
        np.asarray(w["att1"], np.float32), np.asarray(w["Wlin1"], np.float32),
        np.asarray(w["blin1"], np.float32) + np.asarray(w["bias1"], np.float32),
    )
    # layer 2 inputs live in layer-1 permuted space: permute rows by p1 first
    p2, P2, WT2, WR2, c2row, Wlin2p, s2row, inv2 = fold(
        np.asarray(w["Wl2"], np.float32)[p1], np.asarray(w["Wr2"], np.float32)[p1],
        np.asarray(w["bl2"], np.float32), np.asarray(w["br2"], np.float32),
        np.asarray(w["att2"], np.float32), np.asarray(w["Wlin2"], np.float32)[p1],
        np.asarray(w["blin2"], np.float32) + np.asarray(w["bias2"], np.float32),
    )

    shared = {
        "WT1": WT1.astype(ml_dtypes.bfloat16),
        "WR1": WR1.astype(ml_dtypes.bfloat16),
        "Wlin1": Wlin1p.astype(ml_dtypes.bfloat16),
        "WT2": WT2.astype(ml_dtypes.bfloat16),
        "WR2": WR2.astype(ml_dtypes.bfloat16),
        "Wlin2": Wlin2p.astype(ml_dtypes.bfloat16),
        "c1_rep": np.tile(c1row, (128, 1)).astype(np.float32),
        "s1_rep": np.tile(s1row, (128, 1)).astype(np.float32),
        "c2_rep": np.tile(c2row, (128, 1)).astype(np.float32),
        "s2_rep": np.tile(s2row, (128, 1)).astype(np.float32),
        "inv1_rep": np.tile(inv1, (128, 1)).astype(np.float32),
        "inv2_rep": np.tile(inv2, (128, 1)).astype(np.float32),
        "iota_col": np.arange(128, dtype=np.float32)[:, None],
        "iota_row": np.tile(np.arange(128, dtype=ml_dtypes.bfloat16), (128, 1)),
    }
    in_maps = []
    for c in range(NC):
        srcidx = np.zeros((NBLK, 128, T_b), np.int32)
        drel_row = np.full((NBLK, 1, TW), -1.0, np.float32)
        drel_col = np.full((NBLK, 128, T_b), -1.0, np.float32)
        for b, (s_b, drel_b) in enumerate(per_core_blocks[c]):
            n = len(s_b)
            sp = np.zeros(TW, np.int32)
            dp = np.full(TW, -1.0, np.float32)
            sp[:n] = s_b
            dp[:n] = drel_b
            srcidx[b] = sp.reshape(T_b, 128).T
            drel_col[b] = dp.reshape(T_b, 128).T
            drel_row[b, 0] = dp
        im = dict(shared)
        im["x_ownT"] = np.ascontiguousarray(
            xpad[c * NOWN : (c + 1) * NOWN].T
        ).astype(ml_dtypes.bfloat16)
        im["srcidx"] = srcidx
        im["drel_row"] = drel_row.astype(ml_dtypes.bfloat16)
        im["drel_col"] = drel_col
        xg = np.empty((NBLK, D, TW), ml_dtypes.bfloat16)
        for b in range(NBLK):
            flat = srcidx[b].T.ravel()              # edge order e = t*128+p
            xg[b] = xT_b16[:, flat]
        im["xgT"] = xg
        in_maps.append(im)
    return in_maps, T_b, (P1, P2, p2)


def _build_program(T_b, P1, P2):
    TW = T_b * 128
    NBT = NPAD // 128
    nc = bass.Bass()

    def din(name, shape, dt=BF16):
        return nc.dram_tensor(name, shape, dt, kind="ExternalInput")

    x_ownT = din("x_ownT", [D, NOWN])
    xgT = din("xgT", [NBLK, D, TW])
    srcidx = din("srcidx", [NBLK, 128, T_b], I32)
    drel_row = din("drel_row", [NBLK, 1, TW])
    drel_col = din("drel_col", [NBLK, 128, T_b], F32)
    WT1 = din("WT1", [D, F1 + 1]); WR1 = din("WR1", [D, F1]); Wlin1 = din("Wlin1", [D, F1])
    WT2 = din("WT2", [H, F2 + 1]); WR2 = din("WR2", [H, F2]); Wlin2 = din("Wlin2", [H, F2])
    c1_rep = din("c1_rep", [128, F1], F32); s1_rep = din("s1_rep", [128, F1], F32)
    c2_rep = din("c2_rep", [128, F2], F32); s2_rep = din("s2_rep", [128, F2], F32)
    inv1_rep = din("inv1_rep", [128, F1], F32); inv2_rep = din("inv2_rep", [128, F2], F32)
    iota_col = din("iota_col", [128, 1], F32)
    iota_row = din("iota_row", [128, 128])
    out_own = nc.dram_tensor("out_own", [NOWN, O], F32, kind="ExternalOutput")

    with tile.TileContext(nc) as tc:
        with (
            tc.tile_pool(name="dram", bufs=1, space="DRAM") as dram,
            tc.tile_pool(name="const", bufs=1) as cpool,
            tc.tile_pool(name="res", bufs=1) as rpool,
            tc.tile_pool(name="blk", bufs=2) as bpool,
            tc.tile_pool(name="wrt", bufs=2) as wpool,
            tc.tile_pool(name="oh", bufs=2) as ohpool,
            tc.tile_pool(name="g", bufs=2) as gpool,
            tc.tile_pool(name="ga", bufs=4) as gapool,
            tc.tile_pool(name="ix", bufs=3) as ixpool,
            tc.tile_pool(name="ew", bufs=3) as epool,
            tc.tile_pool(name="sg", bufs=3) as segpool,
            tc.tile_pool(name="sm", bufs=2) as smpool,
            tc.tile_pool(name="psr", bufs=2, space="PSUM") as ppsr,
            tc.tile_pool(name="psx", bufs=2, space="PSUM") as ppsx,
            tc.tile_pool(name="pU", bufs=2, space="PSUM") as pU,
            tc.tile_pool(name="pmx", bufs=2, space="PSUM") as pmix,
        ):
            xl2_own_d = dram.tile([NOWN, F2 + 1], BF16)
            table2 = dram.tile([NPAD, F2 + 1], BF16)

            def ld(shape, apsrc, name, dt=BF16):
                t = cpool.tile(shape, dt, tag=name)
                nc.sync.dma_start(out=t[:], in_=apsrc)
                return t

            WT1_s = ld([D, F1 + 1], WT1[:], "WT1")
            WR1_s = ld([D, F1], WR1[:], "WR1")
            Wlin1_s = ld([D, F1], Wlin1[:], "Wlin1")
            WT2_s = ld([H, F2 + 1], WT2[:], "WT2")
            WR2_s = ld([H, F2], WR2[:], "WR2")
            Wlin2_s = ld([H, F2], Wlin2[:], "Wlin2")
            c1_s = ld([128, F1], c1_rep[:], "c1", F32)
            s1_s = ld([128, F1], s1_rep[:], "s1", F32)
            c2_s = ld([128, F2], c2_rep[:], "c2", F32)
            s2_s = ld([128, F2], s2_rep[:], "s2", F32)
            i1_s = ld([128, F1], inv1_rep[:], "inv1", F32)
            i2_s = ld([128, F2], inv2_rep[:], "inv2", F32)
            ic_s = ld([128, 1], iota_col[:], "ic", F32)
            ir_s = ld([128, 128], iota_row[:], "ir")
            ones1 = cpool.tile([1, 128], BF16, tag="ones1")
            nc.vector.memset(ones1[:], 1.0)
            ident = cpool.tile([128, 128], BF16, tag="ident")
            make_identity(nc, ident[:])


            # ---- R1 (-B) and skip for owned blocks ----
            R1_all = rpool.tile([128, NBLK * F1], BF16, tag="R1_all")
            skip_all = rpool.tile([128, NBLK * F1], BF16, tag="skip_all")
            hT_all = rpool.tile([128, NBLK * 128], BF16, tag="hT_all")
            for b in range(NBLK):
                xob = bpool.tile([D, 128], BF16, tag="xob")
                nc.sync.dma_start(out=xob[:], in_=x_ownT[:, b * 128 : (b + 1) * 128])
                ps = pmix.tile([128, F1 + 1], F32, tag="pmx")
                nc.tensor.matmul(out=ps[:, :F1], lhsT=xob[:], rhs=WR1_s[:], start=True, stop=True)
                nc.vector.tensor_add(
                    out=R1_all[:, b * F1 : (b + 1) * F1], in0=ps[:, :F1], in1=c1_s[:]
                )
                ps2 = pmix.tile([128, F1 + 1], F32, tag="pmx")
                nc.tensor.matmul(out=ps2[:, :F1], lhsT=xob[:], rhs=Wlin1_s[:], start=True, stop=True)
                nc.vector.tensor_add(
                    out=skip_all[:, b * F1 : (b + 1) * F1], in0=ps2[:, :F1], in1=s1_s[:]
                )

            R2_all = rpool.tile([128, NBLK * F2], BF16, tag="R2_all")
            skip2_all = rpool.tile([128, NBLK * F2], BF16, tag="skip2_all")

            def edge_pass(layer):
                F = F1 if layer == 1 else F2
                FW = F + 2
                P = P1 if layer == 1 else P2
                G = 4 if layer == 1 else 8     # tiles per psum group (G*F = 512)
                table = None if layer == 1 else table2
                R_src = R1_all if layer == 1 else R2_all
                skip_src = skip_all if layer == 1 else skip2_all
                inv_s = i1_s if layer == 1 else i2_s
                for b in range(NBLK):
                    drow = bpool.tile([1, TW], BF16, tag="drow")
                    nc.sync.dma_start(out=drow[:], in_=drel_row[b, :, :])
                    dcol = bpool.tile([128, T_b], F32, tag="dcol")
                    nc.sync.dma_start(out=dcol[:], in_=drel_col[b, :, :])
                    if layer == 2:
                        idxb = ixpool.tile([128, T_b], I32, tag="idxb")
                        nc.sync.dma_start(out=idxb[:], in_=srcidx[b, :, :])
                    else:
                        xgt = gpool.tile([D, TW], BF16, tag="xgt")
                        nc.sync.dma_start(out=xgt[:], in_=xgT[b, :, :])
                    # one-hot dst-expansion matrix oh[d, e] = (dst_e == d)
                    oh = ohpool.tile([128, TW], BF16, tag="oh")
                    for ci, c0 in enumerate(range(0, TW, 512)):
                        cw = min(512, TW - c0)
                        psr = ppsr.tile([128, 512], F32, tag="psr")
                        nc.tensor.matmul(
                            out=psr[:, :cw], lhsT=ones1[:],
                            rhs=drow[:, c0 : c0 + cw], start=True, stop=True,
                        )
                        nc.vector.tensor_scalar(
                            out=oh[:, c0 : c0 + cw], in0=psr[:, :cw],
                            scalar1=ic_s[:, :1], scalar2=None, op0=AL.is_equal,
                        )
                    # batched gather: [gdot | feat(F)] cols per edge; ones at F+1
                    gall = gapool.tile([128, T_b * FW], BF16, tag="gall")
                    g3 = gall[:].rearrange("p (t f) -> p t f", f=FW)
                    nc.vector.memset(g3[:, :, F + 1 : F + 2], 1.0)
                    if layer == 2:
                        for i in range(T_b):
                            nc.gpsimd.indirect_dma_start(
                                out=gall[:, i * FW : i * FW + F + 1], out_offset=None,
                                in_=table[:],
                                in_offset=bass.IndirectOffsetOnAxis(ap=idxb[:, i : i + 1], axis=0),
                            )
                    else:
                        for t0 in range(0, T_b, 3):
                            tq = min(3, T_b - t0)
                            pg = pmix.tile([128, 3 * (F1 + 1)], F32, tag="pmx")
                            for i in range(t0, t0 + tq):
                                nc.tensor.matmul(
                                    out=pg[:, (i - t0) * (F + 1) : (i - t0 + 1) * (F + 1)],
                                    lhsT=xgt[:, i * 128 : (i + 1) * 128],
                                    rhs=WT1_s[:], start=True, stop=True,
                                )
                            dst = g3[:, t0 : t0 + tq, 0 : F + 1]
                            srcv = pg[:, : tq * (F + 1)].rearrange(
                                "p (t f) -> p t f", f=F + 1
                            )
                            nc.scalar.copy(out=dst, in_=srcv)
                    eall = smpool.tile([128, T_b], F32, tag="eall")
                    dbuf = smpool.tile([128, T_b], F32, tag="dbuf")
                    redp = smpool.tile([128, T_b], F32, tag="redp")
                    redn = smpool.tile([128, T_b], F32, tag="redn")
                    for g0 in range(0, T_b, G):
                        q = min(G, T_b - g0)
                        psx = ppsx.tile([128, 512], F32, tag="psx")
                        for i in range(g0, g0 + q):
                            nc.tensor.matmul(
                                out=psx[:, (i - g0) * F : (i - g0 + 1) * F],
                                lhsT=oh[:, i * 128 : (i + 1) * 128],
                                rhs=R_src[:, b * F : (b + 1) * F],
                                start=True, stop=True,
                            )
                        z = epool.tile([128, 512], BF16, tag="z")
                        nc.vector.tensor_tensor(
                            out=z[:, : q * F],
                            in0=g3[:, g0 : g0 + q, 1 : F + 1],
                            in1=psx[:, : q * F], op=AL.max,
                        )
                        z3 = z[:, : q * F].rearrange("p (t f) -> p t f", f=F)
                        nc.vector.tensor_reduce(
                            out=redp[:, g0 : g0 + q], in_=z3[:, :, :P],
                            axis=mybir.AxisListType.X, op=AL.add,
                        )
                        nc.vector.tensor_reduce(
                            out=redn[:, g0 : g0 + q], in_=z3[:, :, P:],
                            axis=mybir.AxisListType.X, op=AL.add,
                        )
                        nc.vector.tensor_sub(
                            out=dbuf[:, g0 : g0 + q],
                            in0=redp[:, g0 : g0 + q], in1=redn[:, g0 : g0 + q],
                        )
                        gcol = g3[:, g0 : g0 + q, 0:1].rearrange("p q one -> p (q one)")
                        nc.vector.scalar_tensor_tensor(
                            out=eall[:, g0 : g0 + q], in0=dbuf[:, g0 : g0 + q],
                            scalar=0.8, in1=gcol, op0=AL.mult, op1=AL.add,
                        )
                    pall = smpool.tile([128, T_b], F32, tag="pall")
                    nc.scalar.activation(out=pall[:], in_=eall[:], func=ACTF.Exp)
                    U = pU.tile([128, F + 1], F32, tag="pU")
                    for i in range(T_b):
                        seg = segpool.tile([128, 128], BF16, tag="seg")
                        nc.vector.tensor_scalar(
                            out=seg[:], in0=ir_s[:], scalar1=dcol[:, i : i + 1],
                            scalar2=pall[:, i : i + 1], op0=AL.is_equal, op1=AL.mult,
                        )
                        nc.tensor.matmul(
                            out=U[:], lhsT=seg[:],
                            rhs=gall[:, i * FW + 1 : i * FW + F + 2],
                            start=(i == 0), stop=(i == T_b - 1),
                        )
                    se = smpool.tile([128, 1], F32, tag="se")
                    nc.vector.tensor_scalar(
                        out=se[:], in0=U[:, F : F + 1], scalar1=1e-30,
                        scalar2=None, op0=AL.add,
                    )
                    r = smpool.tile([128, 1], F32, tag="r")
                    nc.vector.reciprocal(out=r[:], in_=se[:])
                    t1 = epool.tile([128, F], F32, tag="t1")
                    nc.vector.scalar_tensor_tensor(
                        out=t1[:], in0=U[:, :F], scalar=r[:, :1],
                        in1=inv_s[:], op0=AL.mult, op1=AL.mult,
                    )
                    t2 = epool.tile([128, F], F32, tag="t2")
                    nc.vector.tensor_add(
                        out=t2[:], in0=t1[:], in1=skip_src[:, b * F : (b + 1) * F]
                    )
                    if layer == 1:
                        hb = epool.tile([128, F], BF16, tag="hb")
                        nc.scalar.activation(out=hb[:], in_=t2[:], func=ACTF.Relu)
                        pst = pmix.tile([128, 128], BF16, tag="pmx")
                        nc.tensor.transpose(out=pst[:], in_=hb[:], identity=ident[:])
                        nc.scalar.copy(
                            out=hT_all[:, b * 128 : (b + 1) * 128], in_=pst[:]
                        )
                    else:
                        ob = epool.tile([128, F], F32, tag="ob")
                        nc.scalar.copy(out=ob[:], in_=t2[:])
                        nc.sync.dma_start(
                            out=out_own[b * 128 : (b + 1) * 128, :], in_=ob[:]
                        )

            edge_pass(1)

            # ---- layer-2 tables from h ----
            x2w = wpool.tile([128, 2 * (F2 + 1)], BF16, tag="x2w")
            for b in range(NBLK):
                hTb = hT_all[:, b * 128 : (b + 1) * 128]
                ps = pmix.tile([128, F1 + 1], F32, tag="pmx")
                nc.tensor.matmul(out=ps[:, : F2 + 1], lhsT=hTb, rhs=WT2_s[:], start=True, stop=True)
                half = b % 2
                nc.scalar.copy(
                    out=x2w[:, half * (F2 + 1) : (half + 1) * (F2 + 1)],
                    in_=ps[:, : F2 + 1],
                )
                if half == 1:
                    blk0 = b - 1
                    nc.sync.dma_start(
                        out=xl2_own_d[blk0 * 128 : (blk0 + 2) * 128, :].rearrange(
                            "(t p) f -> p t f", t=2
                        ),
                        in_=x2w[:].rearrange("p (t f) -> p t f", f=F2 + 1),
                    )
                    x2w = wpool.tile([128, 2 * (F2 + 1)], BF16, tag="x2w")
                elif b == NBLK - 1:
                    nc.sync.dma_start(
                        out=xl2_own_d[b * 128 : (b + 1) * 128, :],
                        in_=x2w[:, : F2 + 1],
                    )
                ps2 = pmix.tile([128, F1 + 1], F32, tag="pmx")
                nc.tensor.matmul(out=ps2[:, :F2], lhsT=hTb, rhs=WR2_s[:], start=True, stop=True)
                nc.vector.tensor_add(out=R2_all[:, b * F2 : (b + 1) * F2], in0=ps2[:, :F2], in1=c2_s[:])
                ps3 = pmix.tile([128, F1 + 1], F32, tag="pmx")
                nc.tensor.matmul(out=ps3[:, :F2], lhsT=hTb, rhs=Wlin2_s[:], start=True, stop=True)
                nc.vector.tensor_add(out=skip2_all[:, b * F2 : (b + 1) * F2], in0=ps3[:, :F2], in1=s2_s[:])
            nc.gpsimd.collective_compute(
                "AllGather", AL.bypass,
                replica_groups=[list(range(NC))],
                ins=[xl2_own_d[:].opt()],
                outs=[table2[:].opt()],
            )

            edge_pass(2)

    return nc


_W_KEYS = [
    "Wl1", "bl1", "Wr1", "br1", "att1", "bias1", "Wlin1", "blin1",
    "Wl2", "bl2", "Wr2", "br2", "att2", "bias2", "Wlin2", "blin2",
]


def kernel(x, edge_index, **w):
    _install_fixups()
    from concourse.bass_utils import run_bass_kernel_spmd

    w = {k: np.asarray(w[k], np.float32) for k in _W_KEYS}
    in_maps, T_b, (P1, P2, p2) = _host_prep(np.asarray(x), np.asarray(edge_index), w)
    nc = _build_program(T_b, P1, P2)
    last_err = None
    for attempt in range(3):
        try:
            res = run_bass_kernel_spmd(nc, in_maps, core_ids=list(range(NC)))
            break
        except Exception as exc:  # flaky device recovery
            last_err = exc
            print(f"kernel: attempt {attempt} failed: {exc}", file=sys.stderr)
    else:
        raise last_err
    perm_out = np.concatenate(
        [res.results[c]["out_own"] for c in range(NC)], axis=0
    )[:N]
    out = np.empty_like(perm_out)
    out[:, p2] = perm_out
    return out.astype(np.float32)
